# revision 46
# baseline (speedup 1.0000x reference)
"""Single-step LSTM cell (B=131072, E=H=128) on 8 Trainium2 NeuronCores.

Strategy: pure data-parallel over the batch; each core handles 16384 rows
in transposed layout (contraction dim on SBUF partitions, batch on the
free dim). Work is organized gate-major over variable-width segments
(1024 at the head so the first ACTIVATE fires early, 512 at the tail so
the post-matmul drain chain is short), grouped into supersegments for
DMA and DVE-polynomial granularity. x and h are packed into one DRAM
tensor so each segment needs a single input DMA (3D access pattern); c
rides a per-supersegment DMA. At the head, only the transfers the first
ACTIVATE needs are issued immediately; later transfers are gated behind
them with tiny same-tile copies (WAW deps) because the SDMA engines
round-robin between all queued transfers at packet granularity, which
would starve the critical first chunk. Gate pre-activations accumulate
in fp32 [128, w<=2048] PSUM tiles (double buffered, one ACTIVATE per
gate whose per-partition bias operand applies the gate bias for free).
ScalarE is the binding engine (4 gate evals/element at 1 elem/cycle/
lane, ~61us/core minimum), so tanh(c) alternates between the two
drainers: half the segments evaluate it on the DVE as a clamped
degree-5 odd polynomial (completed-square form, only 2x-mode
tensor_tensor and 4x-mode tensor_scalar ops -- scalar_tensor_tensor
only has a 1x uop); the other half defer it into the next segment's
gate ACTIVATE stream ("pend"), and the last segment runs it inline on
ACT so the drain chain after the final matmul is ~2us. The alternation
is load-balance critical: consecutive DVE-poly segments build a DVE
backlog that stalls the ACT stream. Gate order is f, c~, i, o so
m1 = f*c_prev issues after the first ACTIVATE; c = m1+m2 adds are
deferred past the gate loop. A dummy no-bias ACTIVATE right after the
boot barrier prefetches the sigmoid/tanh table set (~1.3us) off the
critical path, and warmup matmuls ramp the PE clock (HAM) while the
first input DMAs are in flight.

Error budget 2e-2 rel-L2; achieved ~1.0e-2 (all-bf16 c path + 50%
poly tanh). fp8 inputs were evaluated and rejected: even x-only e4m3
costs 2.0e-2 end to end. GPSIMD tensor ops were evaluated and rejected:
a [128,2048] two-input tensor_tensor costs ~4.5us on the Q7s and its
consumers head-of-line block the strict-FIFO queues of ACT/DVE.
"""

import numpy as np

B, E, H = 131072, 128, 128
NCORES = 8
BC = B // NCORES        # 16384 batch rows per core
S = 512                 # matmul moving cols (one PSUM bank)
CW = 4096               # chunk stride inside packed xh SBUF tiles

SEGS = [2048, 2048, 2048, 2048, 2048, 2048, 2048, 1536, 512]
assert sum(SEGS) == BC
SUPER = [(0, 1), (1, 3), (3, 5), (5, 7), (7, 8), (8, 9)]
# tanh(c) deferred into the next supersegment's ACT stream; alternates
# with DVE-poly segments so neither engine builds a backlog
ACT_TANH = {2, 4, 6, 7}
# tail segments: tanh(c) on ACT immediately (short drain chain)
ACT_INLINE = {8}
# segments whose m1 = f*c_prev runs on GPSIMD instead of the DVE; the
# c = m1+m2 adds are deferred past the supersegment's gate loop so the
# ~4us GPSIMD latency never head-of-line blocks the strict-FIFO DVE queue
GP_M1 = set()
# c = m1 + m2 via SBUF->SBUF SDMA accumulate (CCE add) instead of a DVE op
DMA_ACCUM = False

_CACHE = {}


def _build_nc():
    import concourse.bacc as bacc
    import concourse.mybir as mybir
    import concourse.tile as tile

    f32 = mybir.dt.float32
    bf = mybir.dt.bfloat16
    AF = mybir.ActivationFunctionType
    ALU = mybir.AluOpType

    nc = bacc.Bacc("TRN2", target_bir_lowering=False, debug=False,
                   num_devices=NCORES)

    xh = nc.dram_tensor("xh", [E, 2 * BC], bf, kind="ExternalInput").ap()
    cT = nc.dram_tensor("cT", [H, BC], bf, kind="ExternalInput").ap()
    W = nc.dram_tensor("W", [E, 4 * H], bf, kind="ExternalInput").ap()
    U = nc.dram_tensor("U", [H, 4 * H], bf, kind="ExternalInput").ap()
    bias = nc.dram_tensor("b", [H, 4], f32, kind="ExternalInput").ap()
    hT_out = nc.dram_tensor("hT_out", [H, BC], bf, kind="ExternalOutput").ap()
    cT_out = nc.dram_tensor("cT_out", [H, BC], bf, kind="ExternalOutput").ap()

    xh3 = xh.rearrange("p (c n) -> p c n", c=2)

    NSEG = len(SEGS)
    OFF = [sum(SEGS[:i]) for i in range(NSEG)]
    NSS = len(SUPER)
    SSOFF = [OFF[a] for a, _ in SUPER]
    SSW = [sum(SEGS[a:b]) for a, b in SUPER]

    # tanh(x) ~ ((SQB2*x^2 - ASQB2)^2 + B2D) * x on [-PR, PR]
    SQB2 = 0.11248462
    ASQB2 = 0.74192809
    B2D = 0.37794151
    PR = 3.0

    # gate index semantics under order f, c~, i, o:
    GF, GC, GI, GO = 0, 1, 2, 3

    with tile.TileContext(nc) as tc:
        with tc.tile_pool(name="cst", bufs=1) as cst, \
             tc.tile_pool(name="xin", bufs=3) as xin, \
             tc.tile_pool(name="cin", bufs=3) as cin, \
             tc.tile_pool(name="ga", bufs=2) as gap, \
             tc.tile_pool(name="tcp", bufs=2) as tcp, \
             tc.tile_pool(name="mw", bufs=1) as mw, \
             tc.tile_pool(name="pp", bufs=1) as pp, \
             tc.tile_pool(name="co", bufs=2) as cop, \
             tc.tile_pool(name="ho", bufs=2) as hop, \
             tc.tile_pool(name="ps", bufs=2, space="PSUM") as ps:

            W_sb = cst.tile([E, 4 * H], bf)
            U_sb = cst.tile([H, 4 * H], bf)
            b_sb = cst.tile([H, 4], f32)

            xh_t = [None] * NSS
            c_t = [None] * NSS

            def alloc_in(ss):
                xh_t[ss] = xin.tile([E, 2 * CW], bf, tag="xh", name=f"xh{ss}")
                c_t[ss] = cin.tile([H, CW], bf, tag="c", name=f"c{ss}")

            def dma_xh_seg(ss, k):
                off, w = OFF[k], SEGS[k]
                loc = off - SSOFF[ss]
                dst = xh_t[ss].rearrange("p (c n) -> p c n", n=CW)
                nc.sync.dma_start(out=dst[:, :, loc:loc + w],
                                  in_=xh3[:, :, off:off + w])

            def dma_c(ss):
                off, w = SSOFF[ss], SSW[ss]
                nc.sync.dma_start(out=c_t[ss][:, :w], in_=cT[:, off:off + w])

            def prefetch(ss):
                if ss < NSS:
                    alloc_in(ss)
                    for k in range(*SUPER[ss]):
                        dma_xh_seg(ss, k)
                    dma_c(ss)

            # PE warmup + ACT table prefetch first so the gating copies
            # below never block them in the GPSIMD FIFO
            wsrc = cst.tile([E, S], bf, name="wsrc")
            nc.gpsimd.memset(wsrc[:], 1.0)
            tbl = cst.tile([H, 8], bf, name="tbl")
            nc.scalar.activation(tbl[:], wsrc[:, 0:8], AF.Sigmoid)
            warm = ps.tile([H, 2048], f32, tag="g")
            for _ in range(8):
                nc.tensor.matmul(warm[:, 0:S], wsrc[:, 0:H], wsrc[:],
                                 start=True, stop=True)

            # head: only the transfers the first ACTIVATE needs go out
            # immediately -- seg0's x chunk, then W, then seg0's h chunk,
            # b, U. Later transfers are gated behind them with tiny
            # same-tile copies (WAW deps) so the SDMA round-robin does not
            # starve the critical first chunks
            alloc_in(0)
            w0 = SEGS[0]
            dst0 = xh_t[0].rearrange("p (c n) -> p c n", n=CW)
            nc.sync.dma_start(out=dst0[:, 0:1, 0:w0], in_=xh3[:, 0:1, 0:w0])
            nc.sync.dma_start(out=W_sb[:], in_=W)
            nc.sync.dma_start(out=dst0[:, 1:2, 0:w0], in_=xh3[:, 1:2, 0:w0])
            nc.sync.dma_start(out=b_sb[:], in_=bias)
            nc.sync.dma_start(out=U_sb[:], in_=U)
            nc.gpsimd.tensor_copy(c_t[0][:, 0:1], xh_t[0][:, 0:1])
            dma_c(0)
            alloc_in(1)
            nc.gpsimd.tensor_copy(xh_t[1][:, 0:1], xh_t[0][:, 0:1])
            nc.gpsimd.tensor_copy(c_t[1][:, 0:1], xh_t[0][:, CW:CW + 1])
            dma_xh_seg(1, 1)
            dma_xh_seg(1, 2)
            dma_c(1)

            pend = None  # (o_gate, co_tile, ho_tile, off, w, loc)

            for ssi in range(NSS):
                prefetch(ssi + 2)
                a, bnd = SUPER[ssi]
                ssoff, ssw = SSOFF[ssi], SSW[ssi]
                xh_sb, c_sb = xh_t[ssi], c_t[ssi]
                co_sb = cop.tile([H, CW], bf, tag="co", name=f"co{ssi}")
                ho_sb = hop.tile([H, CW], bf, tag="ho", name=f"ho{ssi}")
                o_gates = {}
                seg_parts = {}
                my_pend = None

                for k in range(a, bnd):
                    off, w = OFF[k], SEGS[k]
                    loc = off - ssoff
                    ns = w // S
                    x_sl = xh_sb[:, loc:loc + w]
                    h_sl = xh_sb[:, CW + loc:CW + loc + w]
                    gates = [None] * 4
                    for g in range(4):
                        gp = ps.tile([H, 2048], f32, tag="g",
                                     name=f"gp{k}_{g}")[:, :w]
                        Wg = W_sb[:, g * H:(g + 1) * H]
                        Ug = U_sb[:, g * H:(g + 1) * H]
                        for s in range(ns):
                            sl = slice(s * S, (s + 1) * S)
                            nc.tensor.matmul(gp[:, sl], Wg, x_sl[:, sl],
                                             start=True, stop=False)
                        for s in range(ns):
                            sl = slice(s * S, (s + 1) * S)
                            nc.tensor.matmul(gp[:, sl], Ug, h_sl[:, sl],
                                             start=False, stop=True)
                        ab = gap.tile([H, 2048], bf, tag=f"a{g}",
                                      bufs=3 if g == GO else 2,
                                      name=f"a{g}_{k}")
                        func = AF.Tanh if g == GC else AF.Sigmoid
                        nc.scalar.activation(ab[:, :w], gp[:, :w], func,
                                             bias=b_sb[:, g:g + 1])
                        gates[g] = ab
                        if g == GF:
                            # m1 = f*c_prev right after the first ACTIVATE:
                            # offload latency hides behind gates c~, i, o
                            m1 = mw.tile([H, 2048], bf, tag="m1",
                                         name=f"m1_{k}", bufs=3)
                            eng = nc.gpsimd if k in GP_M1 else nc.vector
                            eng.tensor_mul(out=m1[:, :w],
                                           in0=gates[GF][:, :w],
                                           in1=c_sb[:, loc:loc + w])
                        if g == GI and k == a and pend is not None:
                            pw = pend[4]
                            t_prev = tcp.tile([H, 2048], bf, tag="t")
                            nc.scalar.activation(
                                t_prev[:, :pw],
                                pend[1][:, pend[5]:pend[5] + pw], AF.Tanh)

                    m2 = mw.tile([H, 2048], bf, tag="m2", name=f"m2_{k}",
                                 bufs=3)
                    nc.vector.tensor_mul(out=m2[:, :w], in0=gates[GI][:, :w],
                                         in1=gates[GC][:, :w])
                    o_gates[k] = gates[GO]
                    seg_parts[k] = (m1, m2)

                # deferred adds: by now the GPSIMD m1s have had a whole
                # segment of gate ACTIVATEs to finish
                for k in range(a, bnd):
                    off, w = OFF[k], SEGS[k]
                    loc = off - ssoff
                    m1, m2 = seg_parts[k]
                    nc.vector.tensor_add(out=co_sb[:, loc:loc + w],
                                         in0=m1[:, :w], in1=m2[:, :w])
                    if k in ACT_TANH:
                        assert k == bnd - 1
                        my_pend = (o_gates[k], co_sb, ho_sb, off, w, loc)
                    elif k in ACT_INLINE:
                        t_in = tcp.tile([H, 2048], bf, tag="t",
                                        name=f"ti{k}")
                        nc.scalar.activation(t_in[:, :w],
                                             co_sb[:, loc:loc + w], AF.Tanh)
                        nc.vector.tensor_mul(out=ho_sb[:, loc:loc + w],
                                             in0=o_gates[k][:, :w],
                                             in1=t_in[:, :w])
                        nc.sync.dma_start(out=cT_out[:, off:off + w],
                                          in_=co_sb[:, loc:loc + w])
                        nc.sync.dma_start(out=hT_out[:, off:off + w],
                                          in_=ho_sb[:, loc:loc + w])

                if pend is not None:
                    pa3, pco, pho, poff, pw, ploc = pend
                    nc.vector.tensor_mul(out=pho[:, ploc:ploc + pw],
                                         in0=pa3[:, :pw], in1=t_prev[:, :pw])
                    nc.sync.dma_start(out=hT_out[:, poff:poff + pw],
                                      in_=pho[:, ploc:ploc + pw])
                    pend = None

                inline_w = sum(SEGS[k] for k in range(a, bnd)
                               if k in ACT_INLINE)
                if inline_w < ssw:
                    # c out for the non-inline part of the supersegment
                    nc.sync.dma_start(
                        out=cT_out[:, ssoff:ssoff + ssw - inline_w],
                        in_=co_sb[:, :ssw - inline_w])

                pwid = sum(SEGS[k] for k in range(a, bnd)
                           if k not in ACT_TANH and k not in ACT_INLINE)
                if pwid > 0:
                    xc = pp.tile([H, CW], bf, tag="pc")
                    nc.vector.tensor_scalar(out=xc[:, :pwid],
                                            in0=co_sb[:, :pwid],
                                            scalar1=PR, scalar2=-PR,
                                            op0=ALU.min, op1=ALU.max)
                    u = pp.tile([H, CW], bf, tag="pA", name=f"u{ssi}")
                    nc.vector.tensor_mul(out=u[:, :pwid], in0=xc[:, :pwid],
                                         in1=xc[:, :pwid])
                    wp = pp.tile([H, CW], bf, tag="pB", name=f"w{ssi}")
                    nc.vector.tensor_scalar(out=wp[:, :pwid], in0=u[:, :pwid],
                                            scalar1=SQB2, scalar2=ASQB2,
                                            op0=ALU.mult, op1=ALU.subtract)
                    v = pp.tile([H, CW], bf, tag="pA", name=f"v{ssi}")
                    nc.vector.tensor_mul(out=v[:, :pwid], in0=wp[:, :pwid],
                                         in1=wp[:, :pwid])
                    y2 = pp.tile([H, CW], bf, tag="pB", name=f"y2{ssi}")
                    nc.vector.tensor_scalar(out=y2[:, :pwid],
                                            in0=v[:, :pwid],
                                            scalar1=B2D, scalar2=None,
                                            op0=ALU.add)
                    ty = pp.tile([H, CW], bf, tag="pA", name=f"ty{ssi}")
                    nc.vector.tensor_mul(out=ty[:, :pwid], in0=y2[:, :pwid],
                                         in1=xc[:, :pwid])
                    for k in range(a, bnd):
                        off, w = OFF[k], SEGS[k]
                        loc = off - ssoff
                        if loc >= pwid:
                            continue
                        nc.vector.tensor_mul(
                            out=ho_sb[:, loc:loc + w],
                            in0=o_gates[k][:, :w],
                            in1=ty[:, loc:loc + w])
                    nc.sync.dma_start(out=hT_out[:, ssoff:ssoff + pwid],
                                      in_=ho_sb[:, :pwid])

                pend = my_pend

            assert pend is None

    nc.compile()
    return nc


def kernel(x, hidden_memory_tm1, Wi, Ui, bi, Wf, Uf, bf, Wog, Uog, bog,
           Wc, Uc, bc, _return_timing=False, _trace=False):
    from concourse.bass_utils import run_bass_kernel_spmd

    if "nc" not in _CACHE:
        _CACHE["nc"] = _build_nc()
    nc = _CACHE["nc"]

    import ml_dtypes
    bf16 = ml_dtypes.bfloat16
    x = np.asarray(x, np.float32)
    hm = np.asarray(hidden_memory_tm1, np.float32)
    # gate order f, c~, i, o (f first so m1 = f*c_prev starts early)
    W = np.concatenate([Wf, Wc, Wi, Wog], axis=1).astype(bf16)
    U = np.concatenate([Uf, Uc, Ui, Uog], axis=1).astype(bf16)
    bcat = np.stack([np.asarray(bf), np.asarray(bc), np.asarray(bi),
                     np.asarray(bog)], axis=1).astype(np.float32)  # [H, 4]

    in_maps = []
    for c in range(NCORES):
        sl = slice(c * BC, (c + 1) * BC)
        xTc = np.ascontiguousarray(x[sl].astype(bf16).T)
        hTc = np.ascontiguousarray(hm[0, sl].astype(bf16).T)
        in_maps.append({
            "xh": np.ascontiguousarray(np.concatenate([xTc, hTc], axis=1)),
            "cT": np.ascontiguousarray(hm[1, sl].astype(bf16).T),
            "W": W, "U": U, "b": bcat,
        })

    res = run_bass_kernel_spmd(nc, in_maps, core_ids=list(range(NCORES)),
                               trace=_trace)

    h = np.concatenate(
        [res.results[c]["hT_out"].T.astype(np.float32)
         for c in range(NCORES)], 0)
    cc = np.concatenate(
        [res.results[c]["cT_out"].T.astype(np.float32)
         for c in range(NCORES)], 0)
    out = np.stack([h, cc])
    if _return_timing:
        return out, res
    return out


# revision 47
# speedup vs baseline: 1.0045x; 1.0045x over previous
"""Single-step LSTM cell (B=131072, E=H=128) on 8 Trainium2 NeuronCores.

Strategy: pure data-parallel over the batch; each core handles 16384 rows
in transposed layout (contraction dim on SBUF partitions, batch on the
free dim). Work is organized gate-major over variable-width segments
(1024 at the head so the first ACTIVATE fires early, 512 at the tail so
the post-matmul drain chain is short), grouped into supersegments for
DMA and DVE-polynomial granularity. x and h are packed into one DRAM
tensor so each segment needs a single input DMA (3D access pattern); c
rides a per-supersegment DMA. At the head, only the transfers the first
ACTIVATE needs are issued immediately; later transfers are gated behind
them with tiny same-tile copies (WAW deps) because the SDMA engines
round-robin between all queued transfers at packet granularity, which
would starve the critical first chunk. Gate pre-activations accumulate
in fp32 [128, w<=2048] PSUM tiles (double buffered, one ACTIVATE per
gate whose per-partition bias operand applies the gate bias for free).
ScalarE is the binding engine (4 gate evals/element at 1 elem/cycle/
lane, ~61us/core minimum), so tanh(c) alternates between the two
drainers: half the segments evaluate it on the DVE as a clamped
degree-5 odd polynomial (completed-square form, only 2x-mode
tensor_tensor and 4x-mode tensor_scalar ops -- scalar_tensor_tensor
only has a 1x uop); the other half defer it into the next segment's
gate ACTIVATE stream ("pend"), and the last segment runs it inline on
ACT so the drain chain after the final matmul is ~2us. The alternation
is load-balance critical: consecutive DVE-poly segments build a DVE
backlog that stalls the ACT stream. Gate order is f, c~, i, o so
m1 = f*c_prev issues after the first ACTIVATE; c = m1+m2 adds are
deferred past the gate loop. A dummy no-bias ACTIVATE right after the
boot barrier prefetches the sigmoid/tanh table set (~1.3us) off the
critical path, and warmup matmuls ramp the PE clock (HAM) while the
first input DMAs are in flight.

Error budget 2e-2 rel-L2; achieved ~1.0e-2 (all-bf16 c path + 50%
poly tanh). fp8 inputs were evaluated and rejected: even x-only e4m3
costs 2.0e-2 end to end. GPSIMD tensor ops were evaluated and rejected:
a [128,2048] two-input tensor_tensor costs ~4.5us on the Q7s and its
consumers head-of-line block the strict-FIFO queues of ACT/DVE.
"""

import numpy as np

B, E, H = 131072, 128, 128
NCORES = 8
BC = B // NCORES        # 16384 batch rows per core
S = 512                 # matmul moving cols (one PSUM bank)
CW = 4096               # chunk stride inside packed xh SBUF tiles

SEGS = [1024, 1024, 2048, 2048, 2048, 2048, 2048, 2048, 1536, 512]
assert sum(SEGS) == BC
SUPER = [(0, 2), (2, 4), (4, 6), (6, 8), (8, 9), (9, 10)]
# tanh(c) deferred into the next supersegment's ACT stream; alternates
# with DVE-poly segments so neither engine builds a backlog
ACT_TANH = {3, 5, 7, 8}
# tail segments: tanh(c) on ACT immediately (short drain chain)
ACT_INLINE = {9}
# segments whose m1 = f*c_prev runs on GPSIMD instead of the DVE; the
# c = m1+m2 adds are deferred past the supersegment's gate loop so the
# ~4us GPSIMD latency never head-of-line blocks the strict-FIFO DVE queue
GP_M1 = set()
# c = m1 + m2 via SBUF->SBUF SDMA accumulate (CCE add) instead of a DVE op
DMA_ACCUM = False

_CACHE = {}


def _build_nc():
    import concourse.bacc as bacc
    import concourse.mybir as mybir
    import concourse.tile as tile

    f32 = mybir.dt.float32
    bf = mybir.dt.bfloat16
    AF = mybir.ActivationFunctionType
    ALU = mybir.AluOpType

    nc = bacc.Bacc("TRN2", target_bir_lowering=False, debug=False,
                   num_devices=NCORES)

    xh = nc.dram_tensor("xh", [E, 2 * BC], bf, kind="ExternalInput").ap()
    cT = nc.dram_tensor("cT", [H, BC], bf, kind="ExternalInput").ap()
    W = nc.dram_tensor("W", [E, 4 * H], bf, kind="ExternalInput").ap()
    U = nc.dram_tensor("U", [H, 4 * H], bf, kind="ExternalInput").ap()
    bias = nc.dram_tensor("b", [H, 4], f32, kind="ExternalInput").ap()
    hT_out = nc.dram_tensor("hT_out", [H, BC], bf, kind="ExternalOutput").ap()
    cT_out = nc.dram_tensor("cT_out", [H, BC], bf, kind="ExternalOutput").ap()

    xh3 = xh.rearrange("p (c n) -> p c n", c=2)

    NSEG = len(SEGS)
    OFF = [sum(SEGS[:i]) for i in range(NSEG)]
    NSS = len(SUPER)
    SSOFF = [OFF[a] for a, _ in SUPER]
    SSW = [sum(SEGS[a:b]) for a, b in SUPER]

    # tanh(x) ~ ((SQB2*x^2 - ASQB2)^2 + B2D) * x on [-PR, PR]
    SQB2 = 0.11248462
    ASQB2 = 0.74192809
    B2D = 0.37794151
    PR = 3.0

    # gate index semantics under order f, c~, i, o:
    GF, GC, GI, GO = 0, 1, 2, 3

    with tile.TileContext(nc) as tc:
        with tc.tile_pool(name="cst", bufs=1) as cst, \
             tc.tile_pool(name="xin", bufs=3) as xin, \
             tc.tile_pool(name="cin", bufs=3) as cin, \
             tc.tile_pool(name="ga", bufs=2) as gap, \
             tc.tile_pool(name="tcp", bufs=2) as tcp, \
             tc.tile_pool(name="mw", bufs=1) as mw, \
             tc.tile_pool(name="pp", bufs=1) as pp, \
             tc.tile_pool(name="co", bufs=2) as cop, \
             tc.tile_pool(name="ho", bufs=2) as hop, \
             tc.tile_pool(name="ps", bufs=2, space="PSUM") as ps:

            W_sb = cst.tile([E, 4 * H], bf)
            U_sb = cst.tile([H, 4 * H], bf)
            b_sb = cst.tile([H, 4], f32)

            xh_t = [None] * NSS
            c_t = [None] * NSS

            def alloc_in(ss):
                xh_t[ss] = xin.tile([E, 2 * CW], bf, tag="xh", name=f"xh{ss}")
                c_t[ss] = cin.tile([H, CW], bf, tag="c", name=f"c{ss}")

            def dma_xh_seg(ss, k):
                off, w = OFF[k], SEGS[k]
                loc = off - SSOFF[ss]
                dst = xh_t[ss].rearrange("p (c n) -> p c n", n=CW)
                nc.sync.dma_start(out=dst[:, :, loc:loc + w],
                                  in_=xh3[:, :, off:off + w])

            def dma_c(ss):
                off, w = SSOFF[ss], SSW[ss]
                nc.sync.dma_start(out=c_t[ss][:, :w], in_=cT[:, off:off + w])

            def prefetch(ss):
                if ss < NSS:
                    alloc_in(ss)
                    for k in range(*SUPER[ss]):
                        dma_xh_seg(ss, k)
                    dma_c(ss)

            # PE warmup + ACT table prefetch first so the gating copies
            # below never block them in the GPSIMD FIFO
            wsrc = cst.tile([E, S], bf, name="wsrc")
            nc.gpsimd.memset(wsrc[:], 1.0)
            tbl = cst.tile([H, 8], bf, name="tbl")
            nc.scalar.activation(tbl[:], wsrc[:, 0:8], AF.Sigmoid)
            warm = ps.tile([H, 2048], f32, tag="g")
            for _ in range(3):
                nc.tensor.matmul(warm[:, 0:S], wsrc[:, 0:H], wsrc[:],
                                 start=True, stop=True)

            # head: only the transfers the first ACTIVATE needs go out
            # immediately -- seg0's x chunk, then W, then seg0's h chunk,
            # b, U. Later transfers are gated behind them with tiny
            # same-tile copies (WAW deps) so the SDMA round-robin does not
            # starve the critical first chunks
            alloc_in(0)
            w0 = SEGS[0]
            dst0 = xh_t[0].rearrange("p (c n) -> p c n", n=CW)
            nc.sync.dma_start(out=dst0[:, 0:1, 0:w0], in_=xh3[:, 0:1, 0:w0])
            nc.sync.dma_start(out=W_sb[:], in_=W)
            nc.sync.dma_start(out=dst0[:, 1:2, 0:w0], in_=xh3[:, 1:2, 0:w0])
            nc.sync.dma_start(out=b_sb[:], in_=bias)
            nc.sync.dma_start(out=U_sb[:], in_=U)
            nc.gpsimd.tensor_copy(xh_t[0][:, w0:w0 + 1], xh_t[0][:, 0:1])
            nc.gpsimd.tensor_copy(c_t[0][:, 0:1], xh_t[0][:, 0:1])
            dma_xh_seg(0, 1)
            dma_c(0)
            alloc_in(1)
            nc.gpsimd.tensor_copy(xh_t[1][:, 0:1], xh_t[0][:, 0:1])
            nc.gpsimd.tensor_copy(c_t[1][:, 0:1], xh_t[0][:, CW:CW + 1])
            dma_xh_seg(1, 2)
            dma_xh_seg(1, 3)
            dma_c(1)

            pend = None  # (o_gate, co_tile, ho_tile, off, w, loc)

            for ssi in range(NSS):
                prefetch(ssi + 2)
                a, bnd = SUPER[ssi]
                ssoff, ssw = SSOFF[ssi], SSW[ssi]
                xh_sb, c_sb = xh_t[ssi], c_t[ssi]
                co_sb = cop.tile([H, CW], bf, tag="co", name=f"co{ssi}")
                ho_sb = hop.tile([H, CW], bf, tag="ho", name=f"ho{ssi}")
                o_gates = {}
                seg_parts = {}
                my_pend = None

                for k in range(a, bnd):
                    off, w = OFF[k], SEGS[k]
                    loc = off - ssoff
                    ns = w // S
                    x_sl = xh_sb[:, loc:loc + w]
                    h_sl = xh_sb[:, CW + loc:CW + loc + w]
                    gates = [None] * 4
                    for g in range(4):
                        gp = ps.tile([H, 2048], f32, tag="g",
                                     name=f"gp{k}_{g}")[:, :w]
                        Wg = W_sb[:, g * H:(g + 1) * H]
                        Ug = U_sb[:, g * H:(g + 1) * H]
                        for s in range(ns):
                            sl = slice(s * S, (s + 1) * S)
                            nc.tensor.matmul(gp[:, sl], Wg, x_sl[:, sl],
                                             start=True, stop=False)
                        for s in range(ns):
                            sl = slice(s * S, (s + 1) * S)
                            nc.tensor.matmul(gp[:, sl], Ug, h_sl[:, sl],
                                             start=False, stop=True)
                        ab = gap.tile([H, 2048], bf, tag=f"a{g}",
                                      bufs=3 if g == GO else 2,
                                      name=f"a{g}_{k}")
                        func = AF.Tanh if g == GC else AF.Sigmoid
                        nc.scalar.activation(ab[:, :w], gp[:, :w], func,
                                             bias=b_sb[:, g:g + 1])
                        gates[g] = ab
                        if g == GF:
                            # m1 = f*c_prev right after the first ACTIVATE:
                            # offload latency hides behind gates c~, i, o
                            m1 = mw.tile([H, 2048], bf, tag="m1",
                                         name=f"m1_{k}", bufs=3)
                            eng = nc.gpsimd if k in GP_M1 else nc.vector
                            eng.tensor_mul(out=m1[:, :w],
                                           in0=gates[GF][:, :w],
                                           in1=c_sb[:, loc:loc + w])
                        if g == GI and k == a and pend is not None:
                            pw = pend[4]
                            t_prev = tcp.tile([H, 2048], bf, tag="t")
                            nc.scalar.activation(
                                t_prev[:, :pw],
                                pend[1][:, pend[5]:pend[5] + pw], AF.Tanh)

                    m2 = mw.tile([H, 2048], bf, tag="m2", name=f"m2_{k}",
                                 bufs=3)
                    nc.vector.tensor_mul(out=m2[:, :w], in0=gates[GI][:, :w],
                                         in1=gates[GC][:, :w])
                    o_gates[k] = gates[GO]
                    seg_parts[k] = (m1, m2)

                # deferred adds: by now the GPSIMD m1s have had a whole
                # segment of gate ACTIVATEs to finish
                for k in range(a, bnd):
                    off, w = OFF[k], SEGS[k]
                    loc = off - ssoff
                    m1, m2 = seg_parts[k]
                    nc.vector.tensor_add(out=co_sb[:, loc:loc + w],
                                         in0=m1[:, :w], in1=m2[:, :w])
                    if k in ACT_TANH:
                        assert k == bnd - 1
                        my_pend = (o_gates[k], co_sb, ho_sb, off, w, loc)
                    elif k in ACT_INLINE:
                        t_in = tcp.tile([H, 2048], bf, tag="t",
                                        name=f"ti{k}")
                        nc.scalar.activation(t_in[:, :w],
                                             co_sb[:, loc:loc + w], AF.Tanh)
                        nc.vector.tensor_mul(out=ho_sb[:, loc:loc + w],
                                             in0=o_gates[k][:, :w],
                                             in1=t_in[:, :w])
                        nc.sync.dma_start(out=cT_out[:, off:off + w],
                                          in_=co_sb[:, loc:loc + w])
                        nc.sync.dma_start(out=hT_out[:, off:off + w],
                                          in_=ho_sb[:, loc:loc + w])

                if pend is not None:
                    pa3, pco, pho, poff, pw, ploc = pend
                    nc.vector.tensor_mul(out=pho[:, ploc:ploc + pw],
                                         in0=pa3[:, :pw], in1=t_prev[:, :pw])
                    nc.sync.dma_start(out=hT_out[:, poff:poff + pw],
                                      in_=pho[:, ploc:ploc + pw])
                    pend = None

                inline_w = sum(SEGS[k] for k in range(a, bnd)
                               if k in ACT_INLINE)
                if inline_w < ssw:
                    # c out for the non-inline part of the supersegment
                    nc.sync.dma_start(
                        out=cT_out[:, ssoff:ssoff + ssw - inline_w],
                        in_=co_sb[:, :ssw - inline_w])

                pwid = sum(SEGS[k] for k in range(a, bnd)
                           if k not in ACT_TANH and k not in ACT_INLINE)
                if pwid > 0:
                    xc = pp.tile([H, CW], bf, tag="pc")
                    nc.vector.tensor_scalar(out=xc[:, :pwid],
                                            in0=co_sb[:, :pwid],
                                            scalar1=PR, scalar2=-PR,
                                            op0=ALU.min, op1=ALU.max)
                    u = pp.tile([H, CW], bf, tag="pA", name=f"u{ssi}")
                    nc.vector.tensor_mul(out=u[:, :pwid], in0=xc[:, :pwid],
                                         in1=xc[:, :pwid])
                    wp = pp.tile([H, CW], bf, tag="pB", name=f"w{ssi}")
                    nc.vector.tensor_scalar(out=wp[:, :pwid], in0=u[:, :pwid],
                                            scalar1=SQB2, scalar2=ASQB2,
                                            op0=ALU.mult, op1=ALU.subtract)
                    v = pp.tile([H, CW], bf, tag="pA", name=f"v{ssi}")
                    nc.vector.tensor_mul(out=v[:, :pwid], in0=wp[:, :pwid],
                                         in1=wp[:, :pwid])
                    y2 = pp.tile([H, CW], bf, tag="pB", name=f"y2{ssi}")
                    nc.vector.tensor_scalar(out=y2[:, :pwid],
                                            in0=v[:, :pwid],
                                            scalar1=B2D, scalar2=None,
                                            op0=ALU.add)
                    ty = pp.tile([H, CW], bf, tag="pA", name=f"ty{ssi}")
                    nc.vector.tensor_mul(out=ty[:, :pwid], in0=y2[:, :pwid],
                                         in1=xc[:, :pwid])
                    for k in range(a, bnd):
                        off, w = OFF[k], SEGS[k]
                        loc = off - ssoff
                        if loc >= pwid:
                            continue
                        nc.vector.tensor_mul(
                            out=ho_sb[:, loc:loc + w],
                            in0=o_gates[k][:, :w],
                            in1=ty[:, loc:loc + w])
                    nc.sync.dma_start(out=hT_out[:, ssoff:ssoff + pwid],
                                      in_=ho_sb[:, :pwid])

                pend = my_pend

            assert pend is None

    nc.compile()
    return nc


def kernel(x, hidden_memory_tm1, Wi, Ui, bi, Wf, Uf, bf, Wog, Uog, bog,
           Wc, Uc, bc, _return_timing=False, _trace=False):
    from concourse.bass_utils import run_bass_kernel_spmd

    if "nc" not in _CACHE:
        _CACHE["nc"] = _build_nc()
    nc = _CACHE["nc"]

    import ml_dtypes
    bf16 = ml_dtypes.bfloat16
    x = np.asarray(x, np.float32)
    hm = np.asarray(hidden_memory_tm1, np.float32)
    # gate order f, c~, i, o (f first so m1 = f*c_prev starts early)
    W = np.concatenate([Wf, Wc, Wi, Wog], axis=1).astype(bf16)
    U = np.concatenate([Uf, Uc, Ui, Uog], axis=1).astype(bf16)
    bcat = np.stack([np.asarray(bf), np.asarray(bc), np.asarray(bi),
                     np.asarray(bog)], axis=1).astype(np.float32)  # [H, 4]

    in_maps = []
    for c in range(NCORES):
        sl = slice(c * BC, (c + 1) * BC)
        xTc = np.ascontiguousarray(x[sl].astype(bf16).T)
        hTc = np.ascontiguousarray(hm[0, sl].astype(bf16).T)
        in_maps.append({
            "xh": np.ascontiguousarray(np.concatenate([xTc, hTc], axis=1)),
            "cT": np.ascontiguousarray(hm[1, sl].astype(bf16).T),
            "W": W, "U": U, "b": bcat,
        })

    res = run_bass_kernel_spmd(nc, in_maps, core_ids=list(range(NCORES)),
                               trace=_trace)

    h = np.concatenate(
        [res.results[c]["hT_out"].T.astype(np.float32)
         for c in range(NCORES)], 0)
    cc = np.concatenate(
        [res.results[c]["cT_out"].T.astype(np.float32)
         for c in range(NCORES)], 0)
    out = np.stack([h, cc])
    if _return_timing:
        return out, res
    return out
